# revision 1
# baseline (speedup 1.0000x reference)
"""Trainium2 Bass kernel for the MoR (mixture-of-recursions) stack.

nn_MoRStack: x = FFN(x); then 3 ACT-routed passes
  p_i = sigmoid(router_i(x)); w = ACT-remainder logic; x = w*FFN(x) + (1-w)*x

Strategy: data-parallel over the 8192 tokens across 8 NeuronCores (1024
tokens/core), weights replicated and resident in SBUF as fp16. All matmuls
run with fp16 inputs (full PE rate) and fp32 PSUM accumulation; the x state,
biases and all router/ACT bookkeeping stay fp32. Everything is kept in a
transposed [H, tok] layout so the two FFN matmuls and the router matmuls
chain on the PE with no transposes; per-token state (p, cum, still, wsum)
lives tokens-on-partitions [128, 8]; the per-token blend weight is broadcast
across partitions with a PE transpose + K=1 outer-product matmul.

Host side: input transpose/cast + output gather + the scalar budget-loss.
"""
import sys

if "/opt/trn_rl_repo" not in sys.path:
    sys.path.insert(0, "/opt/trn_rl_repo")

from contextlib import ExitStack

import numpy as np

import concourse.mybir as mybir
import concourse.tile as tile
from concourse import bacc
from concourse.alu_op_type import AluOpType
from concourse.masks import make_identity

F16 = mybir.dt.float16
F32 = mybir.dt.float32
AF = mybir.ActivationFunctionType

B, T, H = 4, 2048, 1024
DFF = 4096
HC = H // 128     # 8 h-chunks
DC = DFF // 128   # 32 dff-chunks
RM = 512          # router hidden
RC = RM // 128    # 4 router-hidden chunks
NITER = 3
ACT_TAU = 0.99
TARGET_DEPTH = 2.5
BUDGET_WEIGHT = 0.01
N_CORES = 8
NTOK = B * T // N_CORES   # 1024 tokens per core


def build(ntok: int, tok_tile: int = 256, repeat: int = 1):
    """Build the Bass program for one core processing `ntok` tokens."""
    assert ntok % tok_tile == 0 and tok_tile % 128 == 0
    ntile = ntok // tok_tile
    nsub = tok_tile // 128          # 128-token subgroups per tile
    ncol = ntok // 128              # columns of the [128, ncol] state layout

    nc = bacc.Bacc("TRN2", target_bir_lowering=False, debug=False)

    d_x16 = nc.dram_tensor("x16", [128, HC, ntok], F16, kind="ExternalInput").ap()
    d_wb1 = nc.dram_tensor("wb1", [128, HC, DFF], F16, kind="ExternalInput").ap()
    d_wb2 = nc.dram_tensor("wb2", [128, DC, H], F16, kind="ExternalInput").ap()
    d_rw1 = nc.dram_tensor("rw1", [128, NITER, HC, RM], F16, kind="ExternalInput").ap()
    d_rw2 = nc.dram_tensor("rw2", [128, NITER, RC], F16, kind="ExternalInput").ap()
    d_bb1 = nc.dram_tensor("bb1", [128, DC], F32, kind="ExternalInput").ap()
    d_bb2 = nc.dram_tensor("bb2", [128, HC], F32, kind="ExternalInput").ap()
    d_rb1 = nc.dram_tensor("rb1", [128, NITER, RC], F32, kind="ExternalInput").ap()
    d_rb2 = nc.dram_tensor("rb2", [128, NITER], F32, kind="ExternalInput").ap()
    d_y = nc.dram_tensor("y", [128, HC, ntok], F32, kind="ExternalOutput").ap()
    d_wsum = nc.dram_tensor("wsum", [128, ncol], F32, kind="ExternalOutput").ap()

    with tile.TileContext(nc) as tc, ExitStack() as ctx:
        pool = ctx.enter_context(tc.tile_pool(name="main", bufs=1))
        hpool = ctx.enter_context(tc.tile_pool(name="hpool", bufs=DC))
        spool = ctx.enter_context(tc.tile_pool(name="scratch", bufs=2))
        rhpool = ctx.enter_context(tc.tile_pool(name="rh", bufs=RC))
        wbcpool = ctx.enter_context(tc.tile_pool(name="wbc", bufs=ntile))
        pmm1 = ctx.enter_context(tc.tile_pool(name="pmm1", bufs=2, space="PSUM"))
        pmm2 = ctx.enter_context(tc.tile_pool(name="pmm2", bufs=2, space="PSUM"))
        paux = ctx.enter_context(tc.tile_pool(name="paux", bufs=3, space="PSUM"))

        wb1 = pool.tile([128, HC, DFF], F16)
        wb2 = pool.tile([128, DC, H], F16)
        rw1 = pool.tile([128, HC, RM], F16)  # one iteration, streamed
        rw2 = pool.tile([128, NITER, RC], F16)
        bb1 = pool.tile([128, DC], F32)
        bb2 = pool.tile([128, HC], F32)
        rb1 = pool.tile([128, NITER, RC], F32)
        rb2 = pool.tile([128, NITER], F32)
        ident = pool.tile([128, 128], F32)
        ones_row = pool.tile([1, 128], F16)

        x16 = [pool.tile([128, HC, tok_tile], F16, name=f"x16_{t}", tag=f"x16_{t}")
               for t in range(ntile)]
        x32 = [pool.tile([128, HC, tok_tile], F32, name=f"x32_{t}", tag=f"x32_{t}")
               for t in range(ntile)]

        cum = pool.tile([128, ncol], F32)
        still = pool.tile([128, ncol], F32)
        wsum = pool.tile([128, ncol], F32)
        zerocol = pool.tile([128, ncol], F32)
        w_rows = pool.tile([1, ncol * 128], F16)

        # weight DMAs (once); Wb1 in column blocks so pass-0 mm1 starts early
        for mb in range(4):
            nc.sync.dma_start(wb1[:, :, mb * 1024:(mb + 1) * 1024],
                              d_wb1[:, :, mb * 1024:(mb + 1) * 1024])
        for kb in range(4):
            nc.sync.dma_start(wb2[:, kb * 8:(kb + 1) * 8, :],
                              d_wb2[:, kb * 8:(kb + 1) * 8, :])
        nc.sync.dma_start(rw2[:], d_rw2[:])
        nc.sync.dma_start(bb1[:], d_bb1[:])
        nc.sync.dma_start(bb2[:], d_bb2[:])
        nc.sync.dma_start(rb1[:], d_rb1[:])
        nc.sync.dma_start(rb2[:], d_rb2[:])

        make_identity(nc, ident[:])
        nc.vector.memset(ones_row[:], 1.0)
        nc.vector.memset(zerocol[:], 0.0)

        def body_pass(t: int, wbc):
            """One FFN body pass on token tile t. If wbc is None (pass 0),
            write x = body(x); else blend x += w*(body(x) - x)."""
            hts = []
            for m in range(DC):
                ph = pmm1.tile([128, tok_tile], F32, tag="ph")
                for k in range(HC):
                    nc.tensor.matmul(
                        ph[:],
                        wb1[:, k, m * 128:(m + 1) * 128],
                        x16[t][:, k, :],
                        start=(k == 0), stop=(k == HC - 1),
                    )
                ht = hpool.tile([128, tok_tile], F16, tag="ht")
                nc.scalar.activation(ht[:], ph[:], AF.Gelu, bias=bb1[:, m:m + 1])
                hts.append(ht)
            for m2 in range(HC):
                px = pmm2.tile([128, tok_tile], F32, tag="px")
                for k2 in range(DC):
                    nc.tensor.matmul(
                        px[:],
                        wb2[:, k2, m2 * 128:(m2 + 1) * 128],
                        hts[k2][:],
                        start=(k2 == 0), stop=(k2 == DC - 1),
                    )
                xs32 = x32[t][:, m2, :]
                xs16 = x16[t][:, m2, :]
                if wbc is None:
                    nc.scalar.activation(xs32, px[:], AF.Identity,
                                         bias=bb2[:, m2:m2 + 1])
                    nc.vector.tensor_copy(xs16, xs32)
                else:
                    # px = (px + bb2) - x ; px *= w ; x += px  (in-place in PSUM)
                    nc.vector.scalar_tensor_tensor(
                        px[:], px[:], bb2[:, m2:m2 + 1], xs32,
                        op0=AluOpType.add, op1=AluOpType.subtract,
                    )
                    nc.vector.tensor_mul(px[:], px[:], wbc[:])
                    nc.vector.tensor_add(xs32, xs32, px[:])
                    nc.vector.tensor_copy(xs16, xs32)

        def router(i: int):
            """Router iteration i: computes blend weights, updates state."""
            p_t = spool.tile([128, ncol], F32, tag="p_t")
            for t in range(ntile):
                rhs_list = []
                for m in range(RC):
                    pr = paux.tile([128, tok_tile], F32, tag="aux")
                    for k in range(HC):
                        nc.tensor.matmul(
                            pr[:],
                            rw1[:, k, m * 128:(m + 1) * 128],
                            x16[t][:, k, :],
                            start=(k == 0), stop=(k == HC - 1),
                        )
                    rh = rhpool.tile([128, tok_tile], F16, tag="rh")
                    nc.scalar.activation(rh[:], pr[:], AF.Gelu,
                                         bias=rb1[:, i, m:m + 1])
                    rhs_list.append(rh)
                # logit: M=128 tokens on partitions, N=1
                pl = paux.tile([128, nsub], F32, tag="aux")
                for s in range(nsub):
                    for k2 in range(RC):
                        nc.tensor.matmul(
                            pl[:, s:s + 1],
                            rhs_list[k2][:, s * 128:(s + 1) * 128],
                            rw2[:, i, k2:k2 + 1],
                            start=(k2 == 0), stop=(k2 == RC - 1),
                        )
                nc.scalar.activation(
                    p_t[:, t * nsub:(t + 1) * nsub], pl[:], AF.Sigmoid,
                    bias=rb2[:, i:i + 1],
                )
            if i + 1 < NITER:
                # prefetch next iteration's router weights (hidden under body)
                nc.sync.dma_start(rw1[:], d_rw1[:, i + 1, :, :])
            # ACT halting state update, tokens-on-partitions [128, ncol]
            s_t = spool.tile([128, ncol], F32, tag="s_t")
            halt = spool.tile([128, ncol], mybir.dt.int32, tag="halt")
            rem = spool.tile([128, ncol], F32, tag="rem")
            wgt = spool.tile([128, ncol], F32, tag="wgt")
            nc.vector.tensor_add(s_t[:], cum[:], p_t[:])
            nc.vector.tensor_scalar(halt[:], s_t[:], ACT_TAU, None,
                                    op0=AluOpType.is_ge)
            nc.vector.tensor_scalar(rem[:], cum[:], -1.0, 1.0,
                                    op0=AluOpType.mult, op1=AluOpType.add)
            nc.vector.tensor_scalar(rem[:], rem[:], 0.0, None, op0=AluOpType.max)
            nc.vector.select(wgt[:], halt[:], rem[:], p_t[:])
            nc.vector.tensor_mul(wgt[:], wgt[:], still[:])
            nc.vector.tensor_add(wsum[:], wsum[:], wgt[:])
            nc.vector.tensor_add(cum[:], cum[:], wgt[:])
            nc.vector.select(still[:], halt[:], zerocol[:], still[:])
            # transpose each wgt column [128,1] -> w_rows [1, c*128:(c+1)*128]
            for c in range(ncol):
                pt = paux.tile([1, 128], F32, tag="aux")
                nc.tensor.transpose(pt[:], wgt[:, c:c + 1], ident[:])
                nc.vector.tensor_copy(w_rows[0:1, c * 128:(c + 1) * 128], pt[:])
            # broadcast across partitions: wbc = ones^T @ w_row
            wbcs = []
            for t in range(ntile):
                pw = paux.tile([128, tok_tile], F32, tag="aux")
                for s in range(nsub):
                    r = t * nsub + s
                    nc.tensor.matmul(
                        pw[:, s * 128:(s + 1) * 128],
                        ones_row[:],
                        w_rows[0:1, r * 128:(r + 1) * 128],
                        start=True, stop=True,
                    )
                wbc = wbcpool.tile([128, tok_tile], F16, tag="wbc")
                nc.vector.tensor_copy(wbc[:], pw[:])
                wbcs.append(wbc)
            return wbcs

        for _rep in range(repeat):
            for t in range(ntile):
                nc.sync.dma_start(x16[t][:],
                                  d_x16[:, :, t * tok_tile:(t + 1) * tok_tile])
            nc.sync.dma_start(rw1[:], d_rw1[:, 0, :, :])
            nc.vector.memset(cum[:], 0.0)
            nc.vector.memset(still[:], 1.0)
            nc.vector.memset(wsum[:], 1.0)

            for t in range(ntile):
                body_pass(t, None)
            for i in range(NITER):
                wbcs = router(i)
                for t in range(ntile):
                    body_pass(t, wbcs[t])

            for t in range(ntile):
                nc.sync.dma_start(d_y[:, :, t * tok_tile:(t + 1) * tok_tile],
                                  x32[t][:])
            nc.sync.dma_start(d_wsum[:], wsum[:])

    nc.compile()
    return nc


def _prep_weights(Wb1, bb1, Wb2, bb2, Rw1, Rb1, Rw2, Rb2):
    f16, f32 = np.float16, np.float32
    out = {}
    out["wb1"] = np.ascontiguousarray(
        np.asarray(Wb1).reshape(8, 128, DFF).transpose(1, 0, 2).astype(f16))
    out["wb2"] = np.ascontiguousarray(
        np.asarray(Wb2).reshape(32, 128, H).transpose(1, 0, 2).astype(f16))
    out["rw1"] = np.ascontiguousarray(
        np.asarray(Rw1)[:, :H, :].reshape(NITER, 8, 128, RM)
        .transpose(2, 0, 1, 3).astype(f16))
    out["rw2"] = np.ascontiguousarray(
        np.asarray(Rw2)[:, :, 0].reshape(NITER, 4, 128).transpose(2, 0, 1).astype(f16))
    out["bb1"] = np.ascontiguousarray(np.asarray(bb1).reshape(32, 128).T.astype(f32))
    out["bb2"] = np.ascontiguousarray(np.asarray(bb2).reshape(8, 128).T.astype(f32))
    out["rb1"] = np.ascontiguousarray(
        np.asarray(Rb1).reshape(NITER, 4, 128).transpose(2, 0, 1).astype(f32))
    out["rb2"] = np.ascontiguousarray(
        np.broadcast_to(np.asarray(Rb2)[:, 0][None, :], (128, NITER)).astype(f32))
    return out


def _prep_x(x_tok):
    """[ntok, H] fp32 -> x16 [128, 8, ntok] fp16 transposed layout."""
    ntok = x_tok.shape[0]
    return np.ascontiguousarray(
        x_tok.T.reshape(8, 128, ntok).transpose(1, 0, 2).astype(np.float16))


_NC_CACHE = {}


def _get_nc():
    key = (NTOK, 256)
    if key not in _NC_CACHE:
        _NC_CACHE[key] = build(NTOK, tok_tile=256)
    return _NC_CACHE[key]


def kernel(x, Wb1, bb1, Wb2, bb2, Rw1, Rb1, Rw2, Rb2):
    from concourse.bass_utils import run_bass_kernel_spmd

    x = np.asarray(x, dtype=np.float32)
    x_flat = x.reshape(B * T, H)
    w = _prep_weights(Wb1, bb1, Wb2, bb2, Rw1, Rb1, Rw2, Rb2)

    nc = _get_nc()
    in_maps = []
    for c in range(N_CORES):
        m = dict(w)
        m["x16"] = _prep_x(x_flat[c * NTOK:(c + 1) * NTOK])
        in_maps.append(m)

    res = run_bass_kernel_spmd(nc, in_maps, core_ids=list(range(N_CORES)))

    y = np.empty((B * T, H), dtype=np.float32)
    wsum_all = np.empty(B * T, dtype=np.float32)
    for c, out in enumerate(res.results):
        # yT [128, 8, ntok]: [p, hc, t] = y[token t, hc*128+p]
        y[c * NTOK:(c + 1) * NTOK] = out["y"].transpose(2, 1, 0).reshape(NTOK, H)
        wsum_all[c * NTOK:(c + 1) * NTOK] = out["wsum"].T.reshape(NTOK)

    avg_depth = np.float32(wsum_all.mean())
    budget_loss = np.float32(BUDGET_WEIGHT) * np.square(
        np.maximum(avg_depth - np.float32(TARGET_DEPTH), np.float32(0.0)))
    return y.reshape(B, T, H), np.float32(budget_loss)
